# revision 3
# baseline (speedup 1.0000x reference)
"""Trainium2 Bass kernel for nn_Attention_35107062677619.

Dense transformer attention block (B=2, S=2048, D=4096, 32 Q heads / 8 KV
heads, head_dim 128, RoPE, causal mask) tensor-parallel over 8 NeuronCores.

Sharding: each core owns 4 Q heads + their shared KV head (GQA groups align
with cores), computes projections + RoPE + attention for those heads, then
applies its 512 columns of wo to its own attention outputs (partial products
over all 4096 output features) and a per-(batch, q-tile) ReduceScatter sums
the partials and hands core c feature rows [c*512, (c+1)*512) — its slice of
the final output.  The host concatenates the 8 feature slices.

Layout strategy (T = feature-major "transposed" layout [feat, tok]):
 - host feeds tile-contiguous xT blocks and pre-transposed bf16 weight shards
   so every matmul operand is a natural SBUF slice; V's [tok, hd] tiles are
   produced with DMA-xbar transposes (no PE/PSUM involvement).
 - Q/K rows are permuted per head into de-interleaved RoPE order (x0 block /
   x1 block) so RoPE becomes partition-shifted copies + multiplies with
   host-precomputed cos/sin tables (signs baked into the sin table).
 - scores are computed transposed (S_T[k, q]); softmax is max-free (scores
   are O(10) pre-mask), the causal mask is applied post-exp as a multiply by
   exp(mask) (exact for -inf/0 masks), fully-masked score tiles are skipped.
 - softmax denominators: es tiles are pair-summed on DVE, then accumulated by
   an all-ones [128,128] matmul so the sums land replicated across all 128
   PSUM partitions; reciprocal_approx_fast + one tensor_tensor multiply
   normalize the attention output (no gpsimd broadcast, no slow reciprocal).
 - phases are interleaved per batch (proj b0, attn+out-proj b0, proj b1, ...)
   and the output projection runs per (batch, q-tile) right behind attention,
   so the 8 ReduceScatters overlap compute and no phase waits on a gather.
 - PSUM budget: 2x[128,1024] score groups + 2x[128,512] AV + 2x[128,512]
   misc (denominator/out-proj) = exactly 8 banks.
"""

import math
import os

import numpy as np
import ml_dtypes

B = 2
S = 2048
D = 4096
HD = 128
N_HEADS = 32
N_KV = 8
N_CORES = 8
NQH = N_HEADS // N_CORES  # 4 local Q heads
P = 128
SLAB = 512  # token tile (matmul free dim)
KH = D // P  # 32 hidden k-tiles
QKVD = NQH * HD + 2 * HD  # 768 projection output dims
F32 = np.float32
BF16 = ml_dtypes.bfloat16


def _build(nc_cores=N_CORES, s=S):
    """Build the SPMD Bass program (one program, data-parallel over cores)."""
    import concourse.mybir as mybir
    import concourse.tile as tile
    from concourse import bacc

    f32 = mybir.dt.float32
    bf16 = mybir.dt.bfloat16
    EXP = mybir.ActivationFunctionType.Exp

    tok = B * s
    nslab = tok // SLAB  # 8 global slabs
    sslab = s // SLAB  # 4 slabs per batch
    nqt = s // SLAB  # 4 q-tiles per batch
    nkt = s // P  # 16 k-tiles per batch
    spk = SLAB // P  # 4 (128-tiles per 512 tile)
    dimt = QKVD // P  # 6 projection dim-tiles
    nfc = D // P  # 32 output-feature chunks

    nc = bacc.Bacc("TRN2", target_bir_lowering=False, debug=False,
                   num_devices=nc_cores)

    xT = nc.dram_tensor("xT", [KH * nslab * P, SLAB], bf16,
                        kind="ExternalInput")
    wqkvT = nc.dram_tensor("wqkvT", [D, QKVD], bf16, kind="ExternalInput")
    woT = nc.dram_tensor("woT", [NQH * HD, D], bf16, kind="ExternalInput")
    cosq = nc.dram_tensor("cosq", [P, s], bf16, kind="ExternalInput")
    sinq = nc.dram_tensor("sinq", [P, s], bf16, kind="ExternalInput")
    expmaskT = nc.dram_tensor("expmaskT", [nqt * SLAB, SLAB], bf16,
                              kind="ExternalInput")
    outT = nc.dram_tensor("outT", [SLAB, tok], bf16, kind="ExternalOutput")

    xT_r = xT.ap().rearrange("(o p) t -> o p t", p=P)  # [KH*nslab, 128, 512]
    wqkvT_r = wqkvT.ap().rearrange("(o p) q -> p o q", p=P)  # [128, 32, 768]
    woT_r = woT.ap().rearrange("(o p) q -> p o q", p=P)  # [128, 4, 4096]
    expmaskT_r = expmaskT.ap().rearrange("(a p) q -> p a q", p=P)

    nwin = B * nqt  # 8 (batch, q-tile) windows

    with tile.TileContext(nc) as tc:
        with (
            tc.tile_pool(name="persist", bufs=1) as persist,
            tc.tile_pool(name="dram", bufs=1, space="DRAM") as dram,
            tc.tile_pool(name="xa", bufs=6) as xpool,
            tc.tile_pool(name="rt", bufs=4) as rpool,
            tc.tile_pool(name="es", bufs=4) as espool,
            tc.tile_pool(name="pr", bufs=4) as prpool,
            tc.tile_pool(name="osb", bufs=8) as opool,
            tc.tile_pool(name="rr", bufs=2) as rrpool,
            tc.tile_pool(name="psb", bufs=4) as pspool,
            tc.tile_pool(name="psG", bufs=2, space="PSUM") as psG,
            tc.tile_pool(name="psA", bufs=2, space="PSUM") as psA,
            tc.tile_pool(name="psC", bufs=2, space="PSUM") as psC,
        ):
            p_dram = [dram.tile([D, SLAB], bf16, tag=f"pd{w}",
                                name=f"pd{w}") for w in range(nwin)]
            rs_out = [dram.tile([SLAB, SLAB], bf16, tag=f"ro{w}",
                                name=f"ro{w}") for w in range(nwin)]

            # ---- persistent SBUF state ----
            wqkv_sb = persist.tile([P, KH, QKVD], bf16, tag="wqkv")
            for c in range(4):
                nc.sync.dma_start(wqkv_sb[:, c * 8:(c + 1) * 8, :],
                                  wqkvT_r[:, c * 8:(c + 1) * 8, :])
            cos_sb = persist.tile([P, s], bf16, tag="cos")
            sin_sb = persist.tile([P, s], bf16, tag="sin")
            nc.sync.dma_start(cos_sb[:], cosq.ap())
            nc.sync.dma_start(sin_sb[:], sinq.ap())
            emask_sb = persist.tile([P, nqt * spk, SLAB], bf16, tag="emask")
            nc.sync.dma_start(emask_sb[:], expmaskT_r)
            wo_sb = persist.tile([P, NQH, D], bf16, tag="wo")
            nc.sync.dma_start(wo_sb[:], woT_r)
            ones_sb = persist.tile([P, P], bf16, tag="ones")
            nc.vector.memset(ones_sb[:], 1.0)

            QT = persist.tile([P, NQH, tok], bf16, tag="QT")
            KT = persist.tile([P, tok], bf16, tag="KT")
            V = persist.tile([P, tok // P, HD], bf16, tag="V")

            for b in range(B):
                # ---- Phase A(b): fused QKV projection + RoPE + V ----
                for sl in range(sslab):
                    slab = b * sslab + sl
                    sr = sl * SLAB
                    t0 = slab * SLAB
                    # 6 dim-tiles: 2x [128,1024] groups + 2x [128,512]
                    g0 = psG.tile([P, 2 * SLAB], f32, tag="g",
                                  name=f"ag0_{slab}")
                    g1 = psG.tile([P, 2 * SLAB], f32, tag="g",
                                  name=f"ag1_{slab}")
                    a0 = psA.tile([P, SLAB], f32, tag="a", name=f"aa0_{slab}")
                    a1 = psA.tile([P, SLAB], f32, tag="a", name=f"aa1_{slab}")
                    dsts = [g0[:, 0:SLAB], g0[:, SLAB:2 * SLAB],
                            g1[:, 0:SLAB], g1[:, SLAB:2 * SLAB],
                            a0[:], a1[:]]
                    for kb in range(KH):
                        xt = xpool.tile([P, SLAB], bf16, tag="x",
                                        name=f"x_{slab}_{kb}")
                        nc.sync.dma_start(xt[:], xT_r[kb * nslab + slab])
                        for d in range(dimt):
                            nc.tensor.matmul(
                                dsts[d],
                                wqkv_sb[:, kb, d * P:(d + 1) * P],
                                xt[:],
                                start=(kb == 0), stop=(kb == KH - 1),
                            )
                    cs = cos_sb[:, sr:sr + SLAB]
                    sn = sin_sb[:, sr:sr + SLAB]
                    for d in range(NQH + 1):
                        dst = (QT[:, d, t0:t0 + SLAB] if d < NQH
                               else KT[:, t0:t0 + SLAB])
                        h = P // 2
                        q_sb = rpool.tile([P, SLAB], bf16, tag="rt",
                                          name=f"qsb_{slab}_{d}")
                        if d % 2 == 1:
                            nc.scalar.copy(q_sb[:], dsts[d])
                        else:
                            nc.vector.tensor_copy(q_sb[:], dsts[d])
                        tmp = rpool.tile([P, SLAB], bf16, tag="rt",
                                         name=f"rt_{slab}_{d}")
                        nc.vector.tensor_copy(tmp[0:h, :], q_sb[h:P, :])
                        nc.vector.tensor_copy(tmp[h:P, :], q_sb[0:h, :])
                        nc.vector.tensor_mul(tmp[:], tmp[:], sn)
                        nc.vector.tensor_mul(dst, q_sb[:], cs)
                        nc.vector.tensor_add(dst, dst, tmp[:])
                    # V: copy out of PSUM, then DMA-xbar transpose to [tok,hd]
                    vtmp = prpool.tile([P, SLAB], bf16, tag="pr",
                                       name=f"vt_{slab}")
                    nc.vector.tensor_copy(vtmp[:], a1[:])
                    for j in range(spk):
                        nc.sync.dma_start_transpose(
                            V[:, slab * spk + j, :],
                            vtmp[:, j * P:(j + 1) * P])

                # ---- Phase B+C(b): attention + output projection ----
                for qt in range(nqt):
                    w = b * nqt + qt
                    nkb = (qt + 1) * spk
                    q0 = b * s + qt * SLAB
                    o_tiles = []
                    for l in range(NQH):
                        pfx = f"{b}_{l}_{qt}"
                        av = psA.tile([P, SLAB], f32, tag="a",
                                      name=f"av_{pfx}")
                        rsum = psC.tile([P, SLAB], f32, tag="c",
                                        name=f"rs_{pfx}")
                        ngrp = nkb // 2
                        for g in range(ngrp):
                            kb0 = 2 * g
                            stg = psG.tile([P, 2 * SLAB], f32, tag="g",
                                           name=f"st_{pfx}_{g}")
                            for j in range(2):
                                kb = kb0 + j
                                nc.tensor.matmul(
                                    stg[:, j * SLAB:(j + 1) * SLAB],
                                    KT[:, b * s + kb * P:b * s + (kb + 1) * P],
                                    QT[:, l, q0:q0 + SLAB],
                                    start=True, stop=True,
                                )
                            es = espool.tile([P, 2 * SLAB], bf16, tag="es",
                                             name=f"es_{pfx}_{g}")
                            nc.scalar.activation(es[:], stg[:], EXP)
                            for j in range(2):
                                kb = kb0 + j
                                if kb >= nkb - spk:
                                    jj = kb - (nkb - spk)
                                    nc.vector.tensor_mul(
                                        es[:, j * SLAB:(j + 1) * SLAB],
                                        es[:, j * SLAB:(j + 1) * SLAB],
                                        emask_sb[:, qt * spk + jj, :])
                            # denominator: DVE pair-sum, ones-matmul accum
                            pr = prpool.tile([P, SLAB], bf16, tag="pr",
                                             name=f"pr_{pfx}_{g}")
                            nc.vector.tensor_add(pr[:], es[:, 0:SLAB],
                                                 es[:, SLAB:2 * SLAB])
                            nc.tensor.matmul(
                                rsum[:], ones_sb[:], pr[:],
                                start=(g == 0), stop=(g == ngrp - 1),
                            )
                            for j in range(2):
                                kb = kb0 + j
                                nc.tensor.matmul(
                                    av[:], V[:, b * nkt + kb, :],
                                    es[:, j * SLAB:(j + 1) * SLAB],
                                    start=(kb == 0), stop=(kb == nkb - 1),
                                )
                        rr = rrpool.tile([P, SLAB], f32, tag="rr",
                                         name=f"rr_{pfx}")
                        nc.vector.reciprocal_approx_fast(rr[:], rsum[:])
                        o = opool.tile([P, SLAB], bf16, tag="o",
                                       name=f"o_{pfx}")
                        nc.vector.tensor_mul(o[:], av[:], rr[:])
                        o_tiles.append(o)
                    # output projection for this window (contract local 512)
                    for fc in range(nfc):
                        pc = psC.tile([P, SLAB], f32, tag="c",
                                      name=f"pc_{w}_{fc}")
                        for l in range(NQH):
                            nc.tensor.matmul(
                                pc[:],
                                wo_sb[:, l, fc * P:(fc + 1) * P],
                                o_tiles[l][:],
                                start=(l == 0), stop=(l == NQH - 1),
                            )
                        pb = pspool.tile([P, SLAB], bf16, tag="pb",
                                         name=f"pb_{w}_{fc}")
                        if fc % 2 == 0:
                            nc.vector.tensor_copy(pb[:], pc[:])
                        else:
                            nc.scalar.copy(pb[:], pc[:])
                        nc.sync.dma_start(
                            p_dram[w][fc * P:(fc + 1) * P, :], pb[:])
                    nc.gpsimd.collective_compute(
                        "ReduceScatter",
                        mybir.AluOpType.add,
                        ins=[p_dram[w].opt()],
                        outs=[rs_out[w].opt()],
                        replica_groups=[list(range(nc_cores))],
                    )
                    nc.sync.dma_start(
                        outT.ap()[:, w * SLAB:(w + 1) * SLAB], rs_out[w][:])

    nc.compile()
    return nc


def _prep_inputs(x, wq, wk, wv, wo, freqs_cos, freqs_sin, mask,
                 nc_cores=N_CORES, s=S):
    """Host-side sharding + layout prep. Returns per-core input maps."""
    tok = B * s
    nqt = s // SLAB
    x = np.asarray(x, F32)
    nslab = tok // SLAB
    # tiled layout: block (kb, slab) = x[slab, :, kb, :].T contiguous
    xT = np.ascontiguousarray(
        x.reshape(nslab, SLAB, D // P, P).transpose(2, 0, 3, 1)
    ).astype(BF16).reshape(D // P * nslab * P, SLAB)

    # de-interleave permutation within a head: [x0_0..x0_63, x1_0..x1_63]
    perm = np.concatenate([np.arange(0, HD, 2), np.arange(1, HD, 2)])

    cos = np.asarray(freqs_cos, F32)  # [s, 64]
    sin = np.asarray(freqs_sin, F32)
    cosq = np.ascontiguousarray(
        np.concatenate([cos.T, cos.T], axis=0)).astype(BF16)
    # the shifted partner is multiplied by the DESTINATION row's sin entry:
    # o_top = x0*c - x1*s  -> top rows carry -sin
    # o_bot = x1*c + x0*s  -> bottom rows carry +sin
    sinq = np.ascontiguousarray(
        np.concatenate([-sin.T, sin.T], axis=0)).astype(BF16)

    m = np.asarray(mask, F32).reshape(s, s)
    blocks = []
    for qt in range(nqt):
        blk = m[qt * SLAB:(qt + 1) * SLAB, qt * SLAB:(qt + 1) * SLAB]
        blocks.append(np.exp(blk.T))  # [k, q]
    expmaskT = np.ascontiguousarray(
        np.concatenate(blocks, axis=0)).astype(BF16)

    scale = 1.0 / math.sqrt(HD)
    in_maps = []
    for c in range(nc_cores):
        wq_c = np.asarray(wq, F32)[c * NQH * HD:(c + 1) * NQH * HD]  # [512, D]
        wq_c = (wq_c.reshape(NQH, HD, D)[:, perm, :] * scale).reshape(
            NQH * HD, D)
        wk_c = np.asarray(wk, F32)[c * HD:(c + 1) * HD][perm, :]  # [128, D]
        wv_c = np.asarray(wv, F32)[c * HD:(c + 1) * HD]  # [128, D]
        wqkvT = np.ascontiguousarray(
            np.concatenate([wq_c, wk_c, wv_c], axis=0).T).astype(BF16)
        # wo columns for this core's heads, transposed: [512 local-hd, 4096]
        woT = np.ascontiguousarray(
            np.asarray(wo, F32)[:, c * NQH * HD:(c + 1) * NQH * HD].T
        ).astype(BF16)
        in_maps.append({
            "xT": xT,
            "wqkvT": wqkvT,
            "woT": woT,
            "cosq": cosq,
            "sinq": sinq,
            "expmaskT": expmaskT,
        })
    return in_maps


_NC_CACHE = {}


def _get_nc(nc_cores=N_CORES, s=S):
    key = (nc_cores, s)
    if key not in _NC_CACHE:
        _NC_CACHE[key] = _build(nc_cores, s)
    return _NC_CACHE[key]


def _assemble(results, nc_cores=N_CORES, s=S):
    out = np.empty((B, s, nc_cores * SLAB), dtype=F32)
    for c in range(nc_cores):
        oT = np.asarray(results[c]["outT"], dtype=F32)  # [512, tok]
        out[:, :, c * SLAB:(c + 1) * SLAB] = oT.T.reshape(B, s, SLAB)
    return out


def _run(inputs, trace=False, nc_cores=N_CORES, s=S):
    from concourse.bass_utils import run_bass_kernel_spmd

    nc = _get_nc(nc_cores, s)
    in_maps = _prep_inputs(**inputs, nc_cores=nc_cores, s=s)
    res = run_bass_kernel_spmd(nc, in_maps, core_ids=list(range(nc_cores)),
                               trace=trace)
    return _assemble(res.results, nc_cores, s), res


def kernel(x, wq, wk, wv, wo, freqs_cos, freqs_sin, mask):
    out, _ = _run(dict(x=x, wq=wq, wk=wk, wv=wv, wo=wo,
                       freqs_cos=freqs_cos, freqs_sin=freqs_sin, mask=mask),
                  trace=bool(int(os.environ.get("KERNEL_TRACE", "0"))))
    return out
